# revision 1
# baseline (speedup 1.0000x reference)
"""Trainium2 Bass kernel for nn_DivEncoder (grouped MLP + ELU + L2 norm).

Math (per batch row n):
  xg = x.reshape(D, V); zeta = einsum('duv,dv->du', W1, xg) + b1
  y_d = b2_d + sum_u W2[d,u] * elu(zeta[d,u]);  out = y / max(||y||, eps)

Decomposition on device (m = min(zeta,0), e = exp(m)):
  elu(zeta) = zeta - m + e - 1
  y = c0 + sum_v wlin[d,v] x[d,v] + sum_u W2 e - sum_u W2 m
  c0 = b2 + sum_u W2 b1 - sum_u W2 ;  wlin = sum_u W2[d,u] W1[d,u,:]

Sharding: batch rows across 8 cores (512 rows each); weights replicated.
Host pre-pass: x is cast to fp16 and transposed per core into feature-major
chunk tiles xt[c] = [128 feats, 512 batch]; all fp16 weight tiles are packed
into one DRAM tensor `wall` loaded with 16 block DMAs (vs ~390 small ones).

Per-core dataflow, 64 chunks of 128 features (8 groups of 16 v's):
  - One DMA loads xt[c] directly as feature-major [128, 512] fp16.
  - L1: 4 fp16 matmuls per chunk (K=32 row strips, tile_position packed)
    accumulate z in 2 PSUM tiles [128, 1024]; 4 more zero-padded K=32
    matmuls add b1 (bias-as-matmul keeps the DVE m-pass scalar-free and
    at FD=1024).
  - m = min(z, 0) on DVE (PSUM->SBUF fp16, 2 ops/chunk); e = exp(m) on
    ACT (one FD=2048 op/chunk -- ACT runs ONLY exp in steady state: any
    other ACT work head-of-line blocks its FIFO and loses wall time).
  - L2: fp16 matmuls (+W2 e, -W2 m; M=32 col-tiled lhsT) plus a wlin
    matmul (M=128) accumulate 16 chunks into one PSUM bank at slot
    partitions 32k + 2*(c%16) + i.
  - Per bank (deferred +4/+6 iters): evac (+c0) on DVE, 4 PE transposes
    against a permutation matrix into a borrowed PSUM bank, DVE copies
    into batch-major tiles; tail: row norm (sqrt + reciprocal + one
    Newton step), scale, contiguous DMA out.

HW-tuned via long-loop slope A/Bs (axon wall-clock is too noisy below
~10 s of device time): DVE m-pass is the bound (~165 us busy), ACT exp
~135, PE ~75-110 (tile_position concurrency is real on HW; the local
TimelineSim charges packed matmuls serially and overstates PE).
"""
import sys
sys.path.insert(0, "/opt/trn_rl_repo")

import numpy as np
import ml_dtypes

import concourse.bass as bass
import concourse.bacc as bacc
import concourse.mybir as mybir
import concourse.tile as tile
from concourse import bass_utils

F32 = mybir.dt.float32
F16 = mybir.dt.float16
BF16 = mybir.dt.bfloat16
AL = mybir.AluOpType
AF = mybir.ActivationFunctionType

N, H, D, U, V = 4096, 8192, 512, 64, 16
NCORE = 8
R = N // NCORE          # 512 batch rows per core
CH = H // 128           # 64 chunks
BG = 4                  # bank groups (16 chunks each)
EPS = 1e-12

NKIND = 4               # w1a, wl1, w2e, w2m
WBLK = 4                # chunks per weight-block DMA
NBLK = CH // WBLK       # 16 block DMAs
BLKC = NKIND * WBLK * 128   # 2048 cols per block

_cache = {}
import os as _os
ABLATE_EM = _os.environ.get("ABLATE_EM", "0") == "1"
ABLATE_ME = _os.environ.get("ABLATE_ME", "0") == "1"
ABLATE_L1 = _os.environ.get("ABLATE_L1", "0") == "1"
ABLATE_WL = _os.environ.get("ABLATE_WL", "0") == "1"
ABLATE_MPASS = _os.environ.get("ABLATE_MPASS", "0") == "1"  # static m tile
ABLATE_EXP = _os.environ.get("ABLATE_EXP", "0") == "1"      # static e tile
# Tuned on HW (long-loop slope A/B): per-chunk exp keeps the ACT engine
# saturated (EGRP>1 and any non-exp ACT work jam its FIFO and lose), the
# bias-as-matmul variant shortens the DVE-critical m-pass (FD=1024, no
# per-slice scalar), and the whole m-pass stays on DVE (ACT_MOD huge).
EGRP = int(_os.environ.get("EGRP", "1"))        # chunks per exp op (1/2/4)
ACT_MOD = int(_os.environ.get("ACT_MOD", "999"))
XDMA_SYNC = _os.environ.get("XDMA_SYNC", "0") == "1"
ME_BUFS = int(_os.environ.get("ME_BUFS", str({1: 6, 2: 4, 4: 2}[EGRP])))
NORM_GPS = _os.environ.get("NORM_GPS", "0") == "1"
BIAS_MM = _os.environ.get("BIAS_MM", "1") == "1"
NORM_MAGIC = _os.environ.get("NORM_MAGIC", "0") == "1"
FIN_ACT = _os.environ.get("FIN_ACT", "0") == "1"
ACC_SQ = _os.environ.get("ACC_SQ", "1") == "1"    # fused square+row-sum on ACT
MERGE_CP = _os.environ.get("MERGE_CP", "1") == "1"  # one strided copy per bank
SCALE_ACT = _os.environ.get("SCALE_ACT", "1") == "1"  # final scale on ACT
XTR_BUFS = int(_os.environ.get("XTR_BUFS", "6"))


def _act_chain(c):
    """Chunks whose m-extraction runs on ACT (as q=relu(-zeta)) not DVE.

    Decided per exp-group (chunks of one group share one exp op)."""
    return (c // EGRP) % ACT_MOD == ACT_MOD - 1


def _wview(t_wall, c, kind):
    base = BLKC * (c // WBLK) + WBLK * 128 * kind + 128 * (c % WBLK)
    return t_wall[:, base:base + 128]


def _build(loop_reps=1, loop_all=False):
    nc = bacc.Bacc("TRN2", target_bir_lowering=False, debug=False,
                   enable_asserts=False, num_devices=NCORE)
    ap = {}
    ap["xt"] = nc.dram_tensor("xt", [CH, 128, R], F16, kind="ExternalInput").ap()
    ap["wall"] = nc.dram_tensor("wall", [NBLK, 128, BLKC], F16,
                                kind="ExternalInput").ap()
    ap["smalls"] = nc.dram_tensor("smalls", [128, 388], F32,
                                  kind="ExternalInput").ap()
    if BIAS_MM:
        ap["b1r"] = nc.dram_tensor("b1r", [128, CH * 128], F16,
                                   kind="ExternalInput").ap()
    y_out = nc.dram_tensor("y", [R, D], F32, kind="ExternalOutput").ap()

    with tile.TileContext(nc) as tc:
        _emit(nc, tc, ap, y_out, loop_reps, loop_all)
    nc.compile()
    return nc


def _emit(nc, tc, ap, y_out, loop_reps=1, loop_all=False):
    with (
        tc.tile_pool(name="wres", bufs=1) as wres,
        tc.tile_pool(name="xtr", bufs=XTR_BUFS) as xtr,
        tc.tile_pool(name="me", bufs=ME_BUFS) as mepool,
        tc.tile_pool(name="yfm", bufs=1) as yfm,
        tc.tile_pool(name="zps", bufs=3, space="PSUM") as zps,
        tc.tile_pool(name="yps", bufs=2, space="PSUM") as yps,
        tc.tile_pool(name="sml", bufs=1) as sml,
    ):
        t_wall = wres.tile([128, NBLK * BLKC], F16, tag="wall", name="wall")
        t_small = wres.tile([128, 388], F32, tag="smalls", name="smalls")
        t_stat = None
        if ABLATE_MPASS or ABLATE_EXP:
            t_stat = wres.tile([128, 2048 * EGRP], F16, tag="stat", name="stat")
            nc.gpsimd.memset(t_stat[:], 0.5)
        t_b1r = t_ones = None
        if BIAS_MM:
            t_b1r = wres.tile([128, CH * 128], F16, tag="b1r", name="b1r")
            t_ones = wres.tile([128, 512], F16, tag="ones", name="ones")
            nc.gpsimd.memset(t_ones[:], 0.0)
            for k4 in range(4):
                nc.gpsimd.memset(t_ones[32 * k4:32 * k4 + 1, :], 1.0)

        def load_weights():
            # smalls first: chunk 0's m-pass needs b1 immediately
            nc.gpsimd.dma_start(t_small[:], ap["smalls"][:])
            if BIAS_MM:
                # split so chunk 0's bias row lands quickly
                q = CH * 128 // 4
                for i4 in range(4):
                    nc.gpsimd.dma_start(t_b1r[:, q * i4:q * (i4 + 1)],
                                        ap["b1r"][:, q * i4:q * (i4 + 1)])
            for b in range(NBLK):
                nc.gpsimd.dma_start(t_wall[:, BLKC * b:BLKC * (b + 1)],
                                    ap["wall"][b])

        def b1v(c):
            return t_small[:, 4 * c:4 * c + 4]

        def c0v(b):
            return t_small[:, 256 + b:257 + b]

        t_id = t_small[:, 260:388]

        import contextlib
        loop_cm = tc.For_i(0, loop_reps, 1) if loop_reps > 1 else contextlib.nullcontext()
        if not loop_all:
            load_weights()
        with loop_cm:
            if loop_all:
                load_weights()
            y_banks = {}
            t_yfm = [yfm.tile([128, 512], F32, tag=f"yfm{b}", name=f"yfm{b}")
                     for b in range(BG)]
            t_yTb = yfm.tile([128, 2048], F32, tag="yTb", name="yTb")
            t_yT = [t_yTb[:, 512 * j:512 * (j + 1)] for j in range(4)]

            sched = {}

            def at(it, fn):
                sched.setdefault(it, []).append(fn)

            def make_fin_a(b):
                def fin_a():
                    ybk = y_banks[b]
                    if FIN_ACT:
                        nc.scalar.activation(t_yfm[b][:], ybk[:], AF.Identity,
                                             bias=c0v(b)[:, 0:1])
                    else:
                        nc.vector.tensor_scalar(t_yfm[b][:], ybk[:],
                                                c0v(b)[:, 0:1], None, AL.add)
                    pTg = zps.tile([128, 512], F32, tag="z", name=f"pTg{b}")
                    for j in range(4):
                        nc.tensor.transpose(pTg[:, 128 * j:128 * (j + 1)],
                                            t_yfm[b][:, 128 * j:128 * (j + 1)],
                                            t_id[:])
                    y_banks[b] = None
                    return pTg
                holder = {}

                def run_a():
                    holder["pTg"] = fin_a()

                def run_b():
                    pTg = holder["pTg"]
                    if MERGE_CP:
                        dst = t_yTb[:].rearrange(
                            "p (j g) -> p j g", j=4)[:, :, 128 * b:128 * b + 128]
                        src = pTg[:].rearrange("p (j f) -> p j f", j=4)
                        nc.vector.tensor_copy(dst, src)
                    else:
                        for j in range(4):
                            if FIN_ACT:
                                nc.scalar.copy(
                                    t_yT[j][:, 128 * b:128 * (b + 1)],
                                    pTg[:, 128 * j:128 * (j + 1)])
                            else:
                                nc.vector.tensor_copy(
                                    t_yT[j][:, 128 * b:128 * (b + 1)],
                                    pTg[:, 128 * j:128 * (j + 1)])
                return run_a, run_b

            for c in range(CH + 6):
                if c < CH:
                    b = c // 16
                    cp = c % 16
                    if cp == 0:
                        y_banks[b] = yps.tile([128, 512], F32, tag="ybank",
                                              name=f"ybank{b}")
                    ybank = y_banks[b]
                    if c % EGRP == 0:
                        grp_m = mepool.tile([128, 2048 * EGRP], F16, tag="m",
                                            name=f"m{c}")
                        grp_e = mepool.tile([128, 2048 * EGRP], F16, tag="e",
                                            name=f"e{c}")
                    off = 2048 * (c % EGRP)
                    m_t = grp_m[:, off:off + 2048]
                    e_t = grp_e[:, off:off + 2048]
                    if ABLATE_MPASS:
                        grp_m, m_t = t_stat, t_stat[:, off:off + 2048]
                    if ABLATE_EXP:
                        grp_e, e_t = t_stat, t_stat[:, off:off + 2048]

                    # --- load x chunk: feature-major [128, 512] fp16 direct
                    xfT = xtr.tile([128, 512], F16, tag="xfT", name=f"xfT{c}")
                    if XDMA_SYNC:
                        leng = nc.sync
                    else:
                        leng = nc.scalar if (c % 2 == 0) else nc.sync
                    leng.dma_start(xfT[:], ap["xt"][c])

                    # --- L1: 1 matmul per block (K=32 row strips)
                    zAB = [zps.tile([128, 1024], F32, tag="z", name=f"z{c}_{h}")
                           for h in range(2)]
                    for k in (range(4) if not ABLATE_L1 else []):
                        zsl = zAB[k // 2][:, 512 * (k % 2):512 * (k % 2) + 512]
                        row = slice(32 * k, 32 * k + 32)
                        nc.tensor.matmul(zsl, _wview(t_wall, c, 0)[row, :],
                                         xfT[row, :],
                                         start=True, stop=not BIAS_MM,
                                         tile_position=(32 * k, 0),
                                         skip_group_check=True)
                        if BIAS_MM:
                            # z += b1 x e0 via zero-padded K=32 matmul (frees
                            # DVE/ACT from per-slice bias adds)
                            nc.tensor.matmul(
                                zsl,
                                t_b1r[32 * k:32 * k + 32, 128 * c:128 * (c + 1)],
                                t_ones[32 * k:32 * k + 32, :],
                                start=False, stop=True,
                                tile_position=(32 * k, 0),
                                skip_group_check=True)
                    # --- wlin matmul (M=128, zero-padded lhsT, fp16)
                    if not ABLATE_WL:
                        nc.tensor.matmul(ybank[:, :], _wview(t_wall, c, 1)[:, :],
                                         xfT[:, :],
                                         start=(cp == 0), stop=False,
                                         skip_group_check=True)
                    # --- m pass (DVE min-chain or ACT relu-chain)
                    if BIAS_MM and not (ABLATE_ME or ABLATE_MPASS):
                        # bias already in PSUM: FD=1024 ops, one per z tile
                        for h in range(2):
                            msl = m_t[:, 1024 * h:1024 * h + 1024]
                            if _act_chain(c):
                                nc.scalar.activation(msl, zAB[h][:], AF.Relu,
                                                     scale=-1.0)
                            else:
                                nc.vector.tensor_scalar(msl, zAB[h][:], 0.0,
                                                        None, AL.min)
                    for k in (range(4) if not (ABLATE_ME or ABLATE_MPASS
                                               or BIAS_MM) else []):
                        zsl = zAB[k // 2][:, 512 * (k % 2):512 * (k % 2) + 512]
                        msl = m_t[:, 512 * k:512 * k + 512]
                        if _act_chain(c):
                            # q = relu(-(z + b1)); host packs b1c = -b1 here
                            nc.scalar.activation(msl, zsl, AF.Relu,
                                                 bias=b1v(c)[:, k:k + 1], scale=-1.0)
                        else:
                            nc.vector.tensor_scalar(msl, zsl, b1v(c)[:, k:k + 1],
                                                    0.0, AL.add, AL.min)
                    # --- e pass (ACT), one op per chunk group; exp(-q) for
                    # ACT-chain groups
                    if (not (ABLATE_ME or ABLATE_EXP)
                            and c % EGRP == EGRP - 1):
                        esc = -1.0 if _act_chain(c) else 1.0
                        nc.scalar.activation(grp_e[:], grp_m[:], AF.Exp,
                                             scale=esc)

                    def em_mms(c=c, m_t=m_t, e_t=e_t):
                        b = c // 16
                        ybk = y_banks[b]
                        last_chunk = (c % 16 == 15)
                        if not ABLATE_EM:
                            for k in range(4):
                                esl = e_t[:, 512 * k:512 * k + 512]
                                msl = m_t[:, 512 * k:512 * k + 512]
                                ysl = ybk[32 * k:32 * k + 32, :]
                                nc.tensor.matmul(
                                    ysl, _wview(t_wall, c, 2)[:, 32 * k:32 * k + 32],
                                    esl,
                                    start=False, stop=False,
                                    tile_position=(0, 32 * k), skip_group_check=True)
                                nc.tensor.matmul(
                                    ysl, _wview(t_wall, c, 3)[:, 32 * k:32 * k + 32],
                                    msl,
                                    start=False, stop=(last_chunk and k == 3),
                                    tile_position=(0, 32 * k), skip_group_check=True)
                    # L2 can only run after the group exp is issued
                    EMD = int(_os.environ.get("EM_DEFER", "2"))
                    at(max(c + EMD, (c // EGRP) * EGRP + EGRP), em_mms)
                    if cp == 15:
                        run_a, run_b = make_fin_a(b)
                        FDA = int(_os.environ.get("FIN_DEFER_A", "4"))
                        FDB = int(_os.environ.get("FIN_DEFER_B", "6"))
                        at(c + FDA, run_a)
                        at(c + FDB, run_b)
                for fn in sched.pop(c, []):
                    fn()

            # ---- norm + output (batch-major tiles already in t_yT)
            for j in range(4):
                yT = t_yT[j]
                sq = xtr.tile([128, 512], F32, tag="sq", name=f"sq{j}")
                ss = sml.tile([128, 1], F32, tag=f"ss{j}")
                if ACC_SQ:
                    # fused square + row-sum on ACT (tail: ACT is idle here)
                    nc.scalar.activation(sq[:], yT[:], AF.Square,
                                         accum_out=ss[:])
                elif NORM_GPS:
                    nc.gpsimd.tensor_mul(sq[:], yT[:], yT[:])
                    nc.vector.reduce_sum(ss[:], sq[:], axis=mybir.AxisListType.X)
                else:
                    nc.scalar.activation(sq[:], yT[:], AF.Square)
                    nc.vector.reduce_sum(ss[:], sq[:], axis=mybir.AxisListType.X)
                if NORM_MAGIC:
                    # rsqrt via bit-trick seed + 3 Newton steps, all on DVE
                    # (avoids the ACT sqrt table-set reload each iteration)
                    r0 = sml.tile([128, 1], F32, tag=f"r0{j}")
                    sh = sml.tile([128, 1], mybir.dt.int32, tag=f"sh{j}")
                    nc.vector.tensor_scalar(sh[:], ss[:].bitcast(mybir.dt.int32),
                                            1, None, AL.logical_shift_right)
                    nc.vector.tensor_scalar(sh[:], sh[:], 0, None,
                                            AL.bitwise_not)
                    nc.vector.tensor_scalar(r0[:].bitcast(mybir.dt.int32),
                                            sh[:], 0x5f3759df + 1, None,
                                            AL.add)
                    r1 = sml.tile([128, 1], F32, tag=f"r1{j}")
                    t1 = sml.tile([128, 1], F32, tag=f"t1{j}")
                    cur = r0
                    for it in range(3):
                        nc.vector.tensor_tensor(t1[:], cur[:], cur[:], AL.mult)
                        nc.vector.tensor_tensor(t1[:], t1[:], ss[:], AL.mult)
                        nc.vector.tensor_scalar(t1[:], t1[:], -0.5, 1.5,
                                                AL.mult, AL.add)
                        nxt = r1 if cur is r0 else r0
                        nc.vector.tensor_tensor(nxt[:], cur[:], t1[:], AL.mult)
                        cur = nxt
                    r1 = cur
                else:
                    s = sml.tile([128, 1], F32, tag=f"s{j}")
                    nc.scalar.activation(s[:], ss[:], AF.Sqrt)
                    nc.vector.tensor_scalar(s[:], s[:], float(EPS), None, AL.max)
                    r0 = sml.tile([128, 1], F32, tag=f"r0{j}")
                    nc.vector.reciprocal(r0[:], s[:])
                    t1 = sml.tile([128, 1], F32, tag=f"t1{j}")
                    nc.vector.tensor_tensor(t1[:], r0[:], r0[:], AL.mult)
                    nc.vector.tensor_tensor(t1[:], t1[:], ss[:], AL.mult)
                    nc.vector.tensor_scalar(t1[:], t1[:], -0.5, 1.5, AL.mult, AL.add)
                    r1 = sml.tile([128, 1], F32, tag=f"r1{j}")
                    nc.vector.tensor_tensor(r1[:], r0[:], t1[:], AL.mult)
                if SCALE_ACT:
                    nc.scalar.activation(yT[:], yT[:], AF.Copy, scale=r1[:])
                elif NORM_GPS:
                    nc.gpsimd.tensor_scalar(yT[:], yT[:], r1[:], None, AL.mult)
                else:
                    nc.vector.tensor_scalar(yT[:], yT[:], r1[:], None, AL.mult)
                oeng = nc.sync if XDMA_SYNC else nc.scalar
                oeng.dma_start(y_out[128 * j:128 * (j + 1), :], yT[:])


def _pack_host(W1, b1, W2, b2):
    W1 = W1.astype(np.float32)
    b1 = b1.astype(np.float32)
    W2 = W2.astype(np.float32)
    b2 = b2.astype(np.float32)

    wlin = np.einsum('du,duv->dv', W2.astype(np.float64),
                     W1.astype(np.float64)).astype(np.float32)
    c0 = b2 + (W2 * b1).sum(-1) - W2.sum(-1)

    W1h = W1.astype(np.float16)
    wlh = wlin.astype(np.float16)
    W2f = W2.astype(np.float16)

    w1hi = np.zeros((CH, 128, 128), np.float16)
    wlhi = np.zeros((CH, 128, 128), np.float16)
    w2e = np.zeros((CH, 128, 128), np.float16)
    b1c = np.zeros((CH, 128, 4), np.float32)
    c0s = np.zeros((BG, 128, 1), np.float32)

    for c in range(CH):
        cp = c % 16
        bi = c // 16
        for k in range(4):
            g0 = 8 * c + 2 * k
            g1 = g0 + 1
            w1hi[c, 32 * k:32 * k + 16, 0:64] = W1h[g0].T
            w1hi[c, 32 * k + 16:32 * k + 32, 64:128] = W1h[g1].T
            scol = 32 * k + 2 * cp
            wlhi[c, 32 * k:32 * k + 16, scol] = wlh[g0]
            wlhi[c, 32 * k + 16:32 * k + 32, scol + 1] = wlh[g1]
            w2e[c, 0:64, scol] = W2f[g0]
            w2e[c, 64:128, scol + 1] = W2f[g1]
            b1c[c, 0:64, k] = b1[g0]
            b1c[c, 64:128, k] = b1[g1]
            c0s[bi, scol, 0] = c0[g0]
            c0s[bi, scol + 1, 0] = c0[g1]
    # bias-as-matmul row table (pristine, un-negated b1)
    b1r = np.zeros((128, CH * 128), np.float16)
    for c in range(CH):
        for k in range(4):
            b1r[32 * k, 128 * c:128 * c + 128] = b1c[c, :, k]

    w2m = -w2e
    for c in range(CH):
        if _act_chain(c):
            b1c[c] = -b1c[c]
            w2m[c] = -w2m[c]
    # permutation matrix: transpose output col j (= d-local) <- slot s
    ident = np.zeros((128, 128), dtype=np.float32)
    for cp in range(16):
        for k in range(4):
            for i_ in range(2):
                jcol = 8 * cp + 2 * k + i_
                slot = 32 * k + 2 * cp + i_
                ident[slot, jcol] = 1.0

    # pack all fp16 weight kinds into [NBLK, 128, BLKC]
    kinds = [w1hi, wlhi, w2e, w2m]
    wall = np.zeros((NBLK, 128, BLKC), np.float16)
    for c in range(CH):
        bi, ci = c // WBLK, c % WBLK
        for k, kt in enumerate(kinds):
            off = WBLK * 128 * k + 128 * ci
            wall[bi, :, off:off + 128] = kt[c]

    # pack fp32 smalls: b1c cols 0..255, c0 cols 256..259, ident 260..387
    smalls = np.zeros((128, 388), np.float32)
    for c in range(CH):
        smalls[:, 4 * c:4 * c + 4] = b1c[c]
    for b in range(BG):
        smalls[:, 256 + b] = c0s[b, :, 0]
    smalls[:, 260:388] = ident
    out = {"wall": wall, "smalls": smalls}
    if BIAS_MM:
        out["b1r"] = b1r
    return out


def _pack_x(x):
    """Per-core host pre-pass: cast fp16 + transpose to [CH, 128, R]."""
    xt = np.ascontiguousarray(x.T.astype(np.float16)).reshape(CH, 128, R)
    return xt


def kernel(x, W1, b1, W2, b2):
    x = np.asarray(x, dtype=np.float32)
    packed = _pack_host(np.asarray(W1), np.asarray(b1),
                        np.asarray(W2), np.asarray(b2))
    if "nc" not in _cache:
        _cache["nc"] = _build()
    nc = _cache["nc"]
    in_maps = []
    for i in range(NCORE):
        m = dict(packed)
        m["xt"] = _pack_x(x[i * R:(i + 1) * R])
        in_maps.append(m)
    res = bass_utils.run_bass_kernel_spmd(nc, in_maps, core_ids=list(range(NCORE)))
    out = np.concatenate([res.results[i]["y"] for i in range(NCORE)], axis=0)
    return out.astype(np.float32)

